# revision 18
# baseline (speedup 1.0000x reference)
"""AutoDOAS forward model on 8 TRN2 NeuronCores — pure data-parallel.

Per core: 256 batch rows (2 tiles x 128 partitions).
Pipeline per 128-row tile:
  1. differential = uT.T @ Mmat  (TensorE, contract=18)  -> padded row in DRAM scratch
  2. tiny MLP (TensorE) -> per-row instrument params
  3. per-(row, 128-col-block) window gather from DRAM via indirect_dma_start
     (element-granular offsets) -> exact piecewise-linear resample with
     relu-basis combine (DVE/ACT)
  4. per-row 15-tap Gaussian LSF conv as 15 diagonal matmuls accumulated
     in PSUM (TensorE)
  5. exp / gain / offset / nonlinearity / stray epilogue (ACT + DVE)
"""
import sys

sys.path.insert(0, "/opt/trn_rl_repo")

import numpy as np
from concourse import bass, bacc, mybir, tile
from concourse.bass_utils import run_bass_kernel_spmd
from concourse.masks import make_identity

B, W, G, E, NI, K = 2048, 8192, 8, 128, 16, 15
NCORES = 8
BS = B // NCORES            # 256 rows per core
NT, TP = 2, 128             # tiles per core, rows per tile
BLK = 128                   # output columns per gather block
NBLK = W // BLK             # 64 blocks per row
LWIN = 140                  # gathered window elems per block
CHB = 8                     # blocks per compute chunk
CHW = CHB * BLK             # 1024
NCH = NBLK // CHB           # 8 chunks per tile
PADL, PADR = 176, 180
ROWP = PADL + W + PADR      # padded row pitch in scratch

F32 = mybir.dt.float32
I32 = mybir.dt.int32
AF = mybir.ActivationFunctionType
OP = mybir.AluOpType

_CACHE = {}


def _bcast(ap, reps):
    """[P, 1] -> [P, reps] stride-0 broadcast."""
    return ap.to_broadcast([ap.shape[0], reps])


def _build(wl0, dlam):
    nc = bacc.Bacc(None)

    gasT_e = nc.declare_dram_parameter("gasT", [G, BS], F32, isOutput=False)
    nuisT_e = nc.declare_dram_parameter("nuisT", [G, BS], F32, isOutput=False)
    am_e = nc.declare_dram_parameter("am", [1, BS], F32, isOutput=False)
    oh_e = nc.declare_dram_parameter("onehotT", [NI, BS], F32, isOutput=False)
    emb_e = nc.declare_dram_parameter("embT16", [NI, E], F32, isOutput=False)
    M_e = nc.declare_dram_parameter("Mmat", [G + G + 2, W], F32, isOutput=False)
    w1a_e = nc.declare_dram_parameter("w1a", [E, 64], F32, isOutput=False)
    w1b_e = nc.declare_dram_parameter("w1b", [G, 64], F32, isOutput=False)
    b1r_e = nc.declare_dram_parameter("b1r", [1, 64], F32, isOutput=False)
    w2s_e = nc.declare_dram_parameter("w2s", [65, 7], F32, isOutput=False)
    out_e = nc.declare_dram_parameter("out", [BS, W], F32, isOutput=True)
    dbgp_e = nc.declare_dram_parameter("dbg_p", [BS, 8], F32, isOutput=True)
    dbgR_e = nc.declare_dram_parameter("dbg_R", [BS, W], F32, isOutput=True)
    dbgrho_e = nc.declare_dram_parameter("dbg_rho", [BS, CHW], F32, isOutput=True)
    dbgwin_e = nc.declare_dram_parameter("dbg_win", [BS, CHB * LWIN], F32, isOutput=True)
    dbgphi_e = nc.declare_dram_parameter("dbg_phi", [BS, NBLK], F32, isOutput=True)
    dbgbi_e = nc.declare_dram_parameter("dbg_bi", [BS, NBLK], I32, isOutput=True)

    scr = [nc.dram_tensor(f"scr{t}", [TP, ROWP], F32) for t in range(NT)]

    NM = G + G + 2  # 18 rows of Mmat / uT

    with tile.TileContext(nc) as tc:
        with (
            tc.tile_pool(name="const", bufs=1) as cp,
            tc.tile_pool(name="small", bufs=2) as sp,
            tc.tile_pool(name="prm", bufs=2) as pp,
            tc.tile_pool(name="mid", bufs=2) as mp,
            tc.tile_pool(name="mid1", bufs=1) as mp1,
            tc.tile_pool(name="cmb", bufs=1) as cb,
            tc.tile_pool(name="big", bufs=1) as bigp,
            tc.tile_pool(name="bnc", bufs=3) as bncp,
            tc.tile_pool(name="psA", bufs=2, space="PSUM") as psA,
            tc.tile_pool(name="psS", bufs=1, space="PSUM") as psS,
            tc.tile_pool(name="psC", bufs=2, space="PSUM") as psC,
        ):
            # ---------- constants / inputs ----------
            Msb = cp.tile([NM, W], F32)
            nc.sync.dma_start(Msb[:], M_e[:])
            gasT = cp.tile([G, BS], F32)
            nc.sync.dma_start(gasT[:], gasT_e[:])
            nuisT = cp.tile([G, BS], F32)
            nc.sync.dma_start(nuisT[:], nuisT_e[:])
            am8 = cp.tile([G, BS], F32)
            nc.sync.dma_start(am8[:], am_e[:].partition_broadcast(G))
            oh = cp.tile([NI, BS], F32)
            nc.sync.dma_start(oh[:], oh_e[:])
            emb16 = cp.tile([NI, E], F32)
            nc.sync.dma_start(emb16[:], emb_e[:])
            w1a = cp.tile([E, 64], F32)
            nc.sync.dma_start(w1a[:], w1a_e[:])
            w1b = cp.tile([G, 64], F32)
            nc.sync.dma_start(w1b[:], w1b_e[:])
            b1r = cp.tile([1, 64], F32)
            nc.sync.dma_start(b1r[:], b1r_e[:])
            w2s = cp.tile([65, 7], F32)
            nc.sync.dma_start(w2s[:], w2s_e[:])

            ident = cp.tile([TP, TP], F32)
            make_identity(nc, ident[:])

            xmodf = cp.tile([TP, CHW], F32)  # 0..127 repeated per block
            nc.gpsimd.iota(xmodf[:], [[0, CHB], [1, BLK]], channel_multiplier=0,
                           allow_small_or_imprecise_dtypes=True)
            blkio = cp.tile([TP, NBLK], F32)  # 128*r
            nc.gpsimd.iota(blkio[:], [[BLK, NBLK]], channel_multiplier=0,
                           allow_small_or_imprecise_dtypes=True)
            rowoff = cp.tile([TP, NBLK], I32)  # p*ROWP + PADL
            nc.gpsimd.iota(rowoff[:], [[0, NBLK]], base=PADL,
                           channel_multiplier=ROWP)
            kio = cp.tile([TP, K], F32)  # -7..7
            nc.gpsimd.iota(kio[:], [[1, K]], base=-7, channel_multiplier=0,
                           allow_small_or_imprecise_dtypes=True)

            # uT = [gas*am, gas, am, 1]  [18, BS]
            ga = cp.tile([G, BS], F32)
            nc.vector.tensor_tensor(ga[:], gasT[:], am8[:], OP.mult)
            ones = cp.tile([1, BS], F32)
            nc.vector.memset(ones[:], 1.0)
            uT = cp.tile([NM, BS], F32)
            nc.sync.dma_start(uT[0:G, :], ga[:])
            nc.sync.dma_start(uT[G:2 * G, :], gasT[:])
            nc.sync.dma_start(uT[2 * G:2 * G + 1, :], am8[0:1, :])
            nc.sync.dma_start(uT[2 * G + 1:NM, :], ones[:])
            negk = cp.tile([TP, 3], F32)
            for k in (1, 2, 3):
                nc.vector.memset(negk[:, k - 1:k], float(-k))

            # embT [128, BS] = emb16.T @ onehotT
            embps = psS.tile([E, BS], F32)
            nc.tensor.matmul(embps[:], emb16[:], oh[:], start=True, stop=True)
            embT = cp.tile([E, BS], F32)
            nc.scalar.activation(embT[:], embps[:], AF.Copy)

            for t in range(NT):
                rs = t * TP  # row offset within the core's 256

                # ---------- differential -> DRAM scratch (padded) ----------
                for c in range(16):
                    dps = psA.tile([TP, 512], F32, tag="dps")
                    nc.tensor.matmul(dps[:], uT[:, rs:rs + TP],
                                     Msb[:, 512 * c:512 * (c + 1)],
                                     start=True, stop=True)
                    dbn = bncp.tile([TP, 512], F32, tag="dbn")
                    nc.scalar.activation(dbn[:], dps[:], AF.Copy)
                    nc.sync.dma_start(
                        scr[t][:, PADL + 512 * c:PADL + 512 * (c + 1)], dbn[:])
                    if c == 0:
                        plt = bncp.tile([TP, PADL], F32, tag="plt")
                        nc.vector.tensor_copy(plt[:], _bcast(dbn[:, 0:1], PADL))
                        nc.sync.dma_start(scr[t][:, 0:PADL], plt[:])
                    if c == 15:
                        prt = bncp.tile([TP, PADR], F32, tag="prt")
                        nc.vector.tensor_copy(prt[:], _bcast(dbn[:, 511:512], PADR))
                        nc.sync.dma_start(scr[t][:, PADL + W:ROWP], prt[:])

                # ---------- MLP ----------
                hps = psS.tile([TP, 64], F32, tag="hps")
                nc.tensor.matmul(hps[:], embT[:, rs:rs + TP], w1a[:],
                                 start=True, stop=False)
                nc.tensor.matmul(hps[:], nuisT[:, rs:rs + TP], w1b[:],
                                 start=False, stop=False)
                nc.tensor.matmul(hps[:], ones[:, rs:rs + TP], b1r[:],
                                 start=False, stop=True)
                h = sp.tile([TP, 64], F32, tag="h")
                nc.scalar.activation(h[:], hps[:], AF.Gelu)
                hTp = psS.tile([64, TP], F32, tag="hTp")
                nc.tensor.transpose(hTp[:], h[:], ident[:])
                hT1 = sp.tile([65, TP], F32, tag="hT1")
                nc.scalar.activation(hT1[0:64, :], hTp[:], AF.Copy)
                nc.vector.memset(hT1[64:65, :], 1.0)
                pps = psS.tile([TP, 7], F32, tag="pps")
                nc.tensor.matmul(pps[:], hT1[:], w2s[:], start=True, stop=True)
                pv = sp.tile([TP, 7], F32, tag="pv")
                nc.scalar.activation(pv[:], pps[:], AF.Copy)
                nc.sync.dma_start(dbgp_e[rs:rs + TP, 0:7], pv[:])

                # ---------- per-row params ----------
                gain = pp.tile([TP, 1], F32, tag="gain")
                nc.scalar.activation(gain[:], pv[:, 0:1], AF.Exp)
                nc.scalar.activation(gain[:], gain[:], AF.Ln, bias=1.0)
                nc.vector.tensor_scalar(gain[:], gain[:], 0.001, None, OP.add)
                th2 = pp.tile([TP, 1], F32, tag="th2")
                nc.scalar.activation(th2[:], pv[:, 2:3], AF.Tanh)
                th3 = pp.tile([TP, 1], F32, tag="th3")
                nc.scalar.activation(th3[:], pv[:, 3:4], AF.Tanh)
                a_sl = pp.tile([TP, 1], F32, tag="a_sl")  # ws - 1
                nc.vector.tensor_scalar(a_sl[:], th3[:], 0.005, None, OP.mult)
                ws_s = pp.tile([TP, 1], F32, tag="ws_s")  # wscale
                nc.vector.tensor_scalar(ws_s[:], th3[:], 0.005, 1.0, OP.mult, OP.add)
                tsh = pp.tile([TP, 1], F32, tag="tsh")  # index-space shift
                nc.vector.tensor_scalar(tsh[:], th3[:], 0.005 * wl0 / dlam, None,
                                        OP.mult)
                nc.vector.scalar_tensor_tensor(tsh[:], th2[:], 0.05 / dlam, tsh[:],
                                               OP.mult, OP.add)
                lsf = pp.tile([TP, 1], F32, tag="lsf")
                nc.scalar.activation(lsf[:], pv[:, 4:5], AF.Exp)
                nc.scalar.activation(lsf[:], lsf[:], AF.Ln, bias=1.0)
                nc.vector.tensor_scalar(lsf[:], lsf[:], 0.001, 5.0, OP.add, OP.min)
                nc.vector.tensor_scalar(lsf[:], lsf[:], 0.2, 1e-6, OP.max, OP.add)
                linv = pp.tile([TP, 1], F32, tag="linv")
                nc.vector.reciprocal(linv[:], lsf[:])
                st_s = pp.tile([TP, 1], F32, tag="st_s")
                nc.scalar.activation(st_s[:], pv[:, 5:6], AF.Tanh, scale=0.5)
                nc.vector.tensor_scalar(st_s[:], st_s[:], 0.025, 0.025, OP.mult,
                                        OP.add)
                omst = pp.tile([TP, 1], F32, tag="omst")
                nc.vector.tensor_scalar(omst[:], st_s[:], -1.0, 1.0, OP.mult, OP.add)
                nonl = pp.tile([TP, 1], F32, tag="nonl")
                nc.scalar.activation(nonl[:], pv[:, 6:7], AF.Tanh)
                nc.vector.tensor_scalar(nonl[:], nonl[:], 0.02, None, OP.mult)

                # ---------- Gaussian kernel + diagonal weights ----------
                kern = sp.tile([TP, K], F32, tag="kern")
                nc.vector.tensor_scalar(kern[:], kio[:], linv[:, 0:1], None, OP.mult)
                nc.scalar.activation(kern[:], kern[:], AF.Square)
                nc.scalar.activation(kern[:], kern[:], AF.Exp, scale=-0.5)
                ksum = pp.tile([TP, 1], F32, tag="ksum")
                nc.vector.tensor_reduce(ksum[:], kern[:], mybir.AxisListType.X,
                                        OP.add)
                krec = pp.tile([TP, 1], F32, tag="krec")
                nc.vector.reciprocal(krec[:], ksum[:])
                nc.vector.tensor_scalar(kern[:], kern[:], krec[:, 0:1], None,
                                        OP.mult)
                diag = mp1.tile([TP, K * TP], F32, tag="diag")
                for k in range(K):
                    nc.vector.tensor_tensor(diag[:, TP * k:TP * (k + 1)], ident[:],
                                            _bcast(kern[:, k:k + 1], TP), OP.mult)

                # ---------- gather indices ----------
                pst = sp.tile([TP, NBLK], F32, tag="pst")
                nc.vector.scalar_tensor_tensor(
                    pst[:], blkio[:], ws_s[:, 0:1],
                    _bcast(tsh[:, 0:1], NBLK),
                    OP.mult, OP.add)
                bfl = sp.tile([TP, NBLK], F32, tag="bfl")
                nc.vector.tensor_scalar(bfl[:], pst[:], 170.5, None, OP.add)
                bi = sp.tile([TP, NBLK], I32, tag="bi")
                nc.vector.tensor_copy(bi[:], bfl[:])
                nc.vector.tensor_scalar(bi[:], bi[:], 172, None, OP.subtract)
                bf2 = sp.tile([TP, NBLK], F32, tag="bf2")
                nc.vector.tensor_copy(bf2[:], bi[:])
                phi = sp.tile([TP, NBLK], F32, tag="phi")
                nc.vector.tensor_tensor(phi[:], pst[:], bf2[:], OP.subtract)
                idx = sp.tile([TP, NBLK], I32, tag="idx")
                nc.vector.tensor_tensor(idx[:], rowoff[:], bi[:], OP.add)

                Rpad = bigp.tile([TP, 2 * 8 + W], F32, tag="Rpad")
                nc.sync.dma_start(dbgphi_e[rs:rs + TP, :], phi[:])
                nc.sync.dma_start(dbgbi_e[rs:rs + TP, :], idx[:])

                # ---------- per-chunk gather + resample ----------
                for c in range(NCH):
                    win = mp.tile([TP, CHB * LWIN], F32, tag="win")
                    for rr in range(CHB):
                        rg = CHB * c + rr
                        nc.gpsimd.indirect_dma_start(
                            out=win[:, LWIN * rr:LWIN * (rr + 1)],
                            out_offset=None,
                            in_=scr[t][:],
                            in_offset=bass.IndirectOffsetOnAxis(
                                ap=idx[:, rg:rg + 1], axis=1),
                        )
                    t1 = cb.tile([TP, CHB * LWIN], F32, tag="t1")
                    nc.vector.tensor_tensor(t1[:, 0:CHB * LWIN - 1],
                                            win[:, 1:], win[:, :-1], OP.subtract)
                    d2 = cb.tile([TP, CHB * LWIN], F32, tag="d2")
                    nc.vector.tensor_tensor(d2[:, 1:CHB * LWIN - 1],
                                            t1[:, 1:CHB * LWIN - 1],
                                            t1[:, 0:CHB * LWIN - 2], OP.subtract)

                    rho = cb.tile([TP, CHW], F32, tag="rho")
                    if c == 1:
                        nc.sync.dma_start(dbgwin_e[rs:rs + TP, :], win[:])
                    nc.vector.scalar_tensor_tensor(
                        rho[:].rearrange("p (b x) -> p b x", x=BLK),
                        xmodf[:].rearrange("p (b x) -> p b x", x=BLK),
                        a_sl[:, 0:1],
                        phi[:, CHB * c:CHB * (c + 1)].unsqueeze(-1)
                        .to_broadcast([TP, CHB, BLK]),
                        OP.mult, OP.add)

                    if c == 1:
                        nc.sync.dma_start(dbgrho_e[rs:rs + TP, :], rho[:])

                    def wap(tl, off):
                        return tl[:].rearrange("p (b l) -> p b l", l=LWIN)[
                            :, :, off:off + BLK]

                    # acc = win[x] + t1[x]*rho + sum_k relu(rho-k)*d2[x+k]
                    def c3(ap):
                        return ap.rearrange("p (b x) -> p b x", x=BLK)

                    t0 = cb.tile([TP, CHW], F32, tag="t0")
                    nc.vector.scalar_tensor_tensor(c3(t0[:]), c3(rho[:]), 1.0,
                                                   wap(t1, 0), OP.mult, OP.mult)
                    acc = mp.tile([TP, CHW], F32, tag="acc")
                    nc.vector.tensor_tensor(c3(acc[:]), wap(win, 0), c3(t0[:]),
                                            OP.add)
                    for k in (1, 2, 3):
                        rk = cb.tile([TP, CHW], F32, tag="rk")
                        nc.scalar.activation(rk[:], rho[:], AF.Relu,
                                             bias=negk[:, k - 1:k])
                        pk = cb.tile([TP, CHW], F32, tag="pk")
                        nc.vector.scalar_tensor_tensor(c3(pk[:]), c3(rk[:]), 1.0,
                                                       wap(d2, k), OP.mult, OP.mult)
                        acc2 = mp.tile([TP, CHW], F32, tag="acc")
                        nc.vector.tensor_tensor(acc2[:], acc[:], pk[:], OP.add)
                        acc = acc2
                    nc.vector.tensor_copy(Rpad[:, 8 + CHW * c:8 + CHW * (c + 1)],
                                          acc[:])

                # conv edge pads (replicate)
                nc.vector.tensor_copy(Rpad[:, 1:8],
                                      _bcast(Rpad[:, 8:9], 7))
                nc.vector.tensor_copy(Rpad[:, 8 + W:15 + W],
                                      _bcast(Rpad[:, 7 + W:8 + W], 7))

                nc.sync.dma_start(dbgR_e[rs:rs + TP, :], Rpad[:, 8:8 + W])
                # ---------- conv (PE) + chunked epilogue ----------
                nl = bigp.tile([TP, W], F32, tag="nl")
                msum = sp.tile([TP, 16], F32, tag="msum")
                for c in range(16):
                    cps = psC.tile([TP, 512], F32, tag="cps")
                    for k in range(K):
                        nc.tensor.matmul(cps[:], diag[:, TP * k:TP * (k + 1)],
                                         Rpad[:, 1 + 512 * c + k:1 + 512 * c + k + 512],
                                         start=(k == 0), stop=(k == K - 1))
                    cnt = bncp.tile([TP, 512], F32, tag="cnt")
                    nc.scalar.activation(cnt[:], cps[:], AF.Exp, scale=-1.0)
                    pst_c = bncp.tile([TP, 512], F32, tag="pst_c")
                    nc.scalar.activation(pst_c[:], cnt[:], AF.Identity,
                                         bias=pv[:, 1:2], scale=gain[:, 0:1])
                    sq_c = bncp.tile([TP, 512], F32, tag="sq_c")
                    nc.scalar.activation(sq_c[:], pst_c[:], AF.Square)
                    nc.vector.scalar_tensor_tensor(
                        nl[:, 512 * c:512 * (c + 1)], sq_c[:], nonl[:, 0:1],
                        pst_c[:], OP.mult, OP.add, accum_out=msum[:, c:c + 1])
                mtot = pp.tile([TP, 1], F32, tag="mtot")
                nc.vector.tensor_reduce(mtot[:], msum[:], mybir.AxisListType.X,
                                        OP.add)
                stm = pp.tile([TP, 1], F32, tag="stm")
                nc.vector.tensor_scalar(stm[:], mtot[:], st_s[:, 0:1], 1.0 / W,
                                        OP.mult, OP.mult)
                for c in range(16):
                    fin = bncp.tile([TP, 512], F32, tag="fin")
                    nc.vector.tensor_scalar(fin[:], nl[:, 512 * c:512 * (c + 1)],
                                            omst[:, 0:1], stm[:, 0:1],
                                            OP.mult, OP.add)
                    nc.sync.dma_start(out_e[rs:rs + TP, 512 * c:512 * (c + 1)],
                                      fin[:])

    nc.compile()
    return nc


def _prep(inputs):
    gas = np.asarray(inputs["gas_columns"], np.float32)
    ids = np.asarray(inputs["instrument_ids"]).astype(np.int64)
    nuis = np.asarray(inputs["nuisance_latent"], np.float32)
    am = np.asarray(inputs["air_mass"], np.float32)
    wl = np.asarray(inputs["wavelengths_nm"], np.float32)
    ab = np.asarray(inputs["absorption"], np.float32)
    cb = np.asarray(inputs["continuum_basis"], np.float32)
    ray = np.asarray(inputs["rayleigh_od"], np.float32)
    emb = np.asarray(inputs["embed_table"], np.float32)
    w1 = np.asarray(inputs["w1"], np.float32)
    b1 = np.asarray(inputs["b1"], np.float32)
    w2 = np.asarray(inputs["w2"], np.float32)
    b2 = np.asarray(inputs["b2"], np.float32)

    wl0 = float(wl[0])
    dlam = float((wl[-1] - wl[0]) / (W - 1))

    Mmat = np.concatenate(
        [ab, cb[0:G], ray[None, :], cb[G:G + 1]], axis=0).astype(np.float32)
    w1a = np.ascontiguousarray(w1[G:G + E])            # embed rows
    w1b = np.ascontiguousarray(w1[0:G])                # nuisance rows
    b1r = b1[None, :].astype(np.float32)
    w2s = np.concatenate([w2, b2[None, :]], 0)

    in_maps = []
    for c in range(NCORES):
        r = slice(c * BS, (c + 1) * BS)
        oh = (ids[r][None, :] == np.arange(NI)[:, None]).astype(np.float32)
        in_maps.append({
            "gasT": np.ascontiguousarray(gas[r].T),
            "nuisT": np.ascontiguousarray(nuis[r].T),
            "am": np.ascontiguousarray(am[r][None, :]),
            "onehotT": oh,
            "embT16": emb,
            "Mmat": Mmat,
            "w1a": w1a,
            "w1b": w1b,
            "b1r": b1r,
            "w2s": w2s,
        })
    return in_maps, wl0, dlam


def kernel(**inputs):
    in_maps, wl0, dlam = _prep(inputs)
    key = (round(wl0, 6), round(dlam, 9))
    if key not in _CACHE:
        _CACHE[key] = _build(wl0, dlam)
    nc = _CACHE[key]
    res = run_bass_kernel_spmd(nc, in_maps, core_ids=list(range(NCORES)))
    outs = [res.results[i]["out"] for i in range(NCORES)]
    return np.concatenate(outs, axis=0).astype(np.float32)


if __name__ == "__main__":
    rng = np.random.default_rng(0)
    ins = {
        "gas_columns": rng.random((B, G), dtype=np.float32),
        "instrument_ids": rng.integers(0, NI, B),
        "nuisance_latent": rng.standard_normal((B, G)).astype(np.float32),
        "air_mass": 1 + 2 * rng.random(B, dtype=np.float32),
        "wavelengths_nm": np.linspace(300, 400, W, dtype=np.float32),
        "absorption": 0.1 * rng.random((G, W), dtype=np.float32),
        "continuum_basis": 0.05 * rng.standard_normal((G + 1, W)).astype(np.float32),
        "rayleigh_od": rng.random(W, dtype=np.float32),
        "embed_table": rng.standard_normal((NI, E)).astype(np.float32),
        "w1": rng.standard_normal((G + E, 64)).astype(np.float32) / 12,
        "b1": np.zeros(64, np.float32),
        "w2": rng.standard_normal((64, 7)).astype(np.float32) / 8,
        "b2": np.zeros(7, np.float32),
    }
    out = kernel(**ins)
    print("out", out.shape, out.dtype, np.abs(out).mean())


# revision 58
# speedup vs baseline: 23489.0173x; 23489.0173x over previous
"""AutoDOAS forward model on 8 TRN2 NeuronCores — pure data-parallel.

Per core: 256 batch rows (2 tiles x 128 partitions).
Pipeline per 128-row tile:
  1. differential = uT.T @ Mmat  (TensorE, contract=18)  -> padded row in DRAM scratch
  2. tiny MLP (TensorE) -> per-row instrument params
  3. per-(row, 128-col-block) window gather from DRAM via indirect_dma_start
     (element-granular offsets) -> exact piecewise-linear resample with
     relu-basis combine (DVE/ACT)
  4. per-row 15-tap Gaussian LSF conv as 15 diagonal matmuls accumulated
     in PSUM (TensorE)
  5. exp / gain / offset / nonlinearity / stray epilogue (ACT + DVE)
"""
import sys

sys.path.insert(0, "/opt/trn_rl_repo")

import numpy as np
from concourse import bass, bacc, mybir, tile
from concourse.bass_utils import run_bass_kernel_spmd
from concourse.masks import make_identity

B, W, G, E, NI, K = 2048, 8192, 8, 128, 16, 15
NCORES = 8
BS = B // NCORES            # 256 rows per core
NT, TP = 2, 128             # tiles per core, rows per tile
BLK = 128                   # output columns per gather block
NBLK = W // BLK             # 64 blocks per row
LWIN = 132                  # gathered window elems per block
CHB = 16                    # blocks per compute chunk
CHW = CHB * BLK             # 2048
NCH = NBLK // CHB           # 4 chunks per tile
PADL, PADR = 176, 180
ROWP = PADL + W + PADR      # padded row pitch in scratch (unused w/ segments)
SEGW, SEGE = 2048, 176      # segment width / edge overlap
SEGP = SEGW + 2 * SEGE      # 2400 segment pitch

F32 = mybir.dt.float32
BF16 = mybir.dt.bfloat16
I32 = mybir.dt.int32
AF = mybir.ActivationFunctionType
OP = mybir.AluOpType

_CACHE = {}


def _bcast(ap, reps):
    """[P, 1] -> [P, reps] stride-0 broadcast."""
    return ap.to_broadcast([ap.shape[0], reps])


def _build(wl0, dlam):
    nc = bacc.Bacc(None)

    gasT_e = nc.declare_dram_parameter("gasT", [G, BS], F32, isOutput=False)
    uTp_e = nc.declare_dram_parameter("uTpre", [G + G + 2, BS], F32, isOutput=False)
    amr_e = nc.declare_dram_parameter("amr8", [G, BS], F32, isOutput=False)
    nuisT_e = nc.declare_dram_parameter("nuisT", [G, BS], F32, isOutput=False)
    am_e = nc.declare_dram_parameter("am", [1, BS], F32, isOutput=False)
    oh_e = nc.declare_dram_parameter("onehotT", [NI, BS], F32, isOutput=False)
    emb_e = nc.declare_dram_parameter("embT16", [NI, E], F32, isOutput=False)
    M_e = nc.declare_dram_parameter("Mmat", [G + G + 2, W], F32, isOutput=False)
    Mb_e = nc.declare_dram_parameter("Mbar", [G + G + 2, 1], F32, isOutput=False)
    w1a_e = nc.declare_dram_parameter("w1a", [E, 64], F32, isOutput=False)
    w1b_e = nc.declare_dram_parameter("w1b", [G, 64], F32, isOutput=False)
    b1r_e = nc.declare_dram_parameter("b1r", [1, 64], F32, isOutput=False)
    w2s_e = nc.declare_dram_parameter("w2s", [65, 7], F32, isOutput=False)
    out_e = nc.declare_dram_parameter("out", [BS, W], F32, isOutput=True)

    scr = [[nc.dram_tensor(f"scr{t}_{s}", [TP, SEGP], F32) for s in range(NCH)]
           for t in range(NT)]

    NM = G + G + 2  # 18 rows of Mmat / uT

    with tile.TileContext(nc) as tc:
        with (
            tc.tile_pool(name="const", bufs=1) as cp,
            tc.tile_pool(name="small", bufs=2) as sp,
            tc.tile_pool(name="prm", bufs=2) as pp,
            tc.tile_pool(name="mid", bufs=3) as mp,
            tc.tile_pool(name="mid1", bufs=2) as mp1,
            tc.tile_pool(name="cmb", bufs=1) as cb,
            tc.tile_pool(name="big", bufs=2) as bigp,
            tc.tile_pool(name="big2", bufs=1) as bigp2,
            tc.tile_pool(name="bnc", bufs=2) as bncp,
            tc.tile_pool(name="psA", bufs=2, space="PSUM") as psA,
            tc.tile_pool(name="psS", bufs=1, space="PSUM") as psS,
            tc.tile_pool(name="psC", bufs=3, space="PSUM") as psC,
        ):
            # ---------- constants / inputs ----------
            gasT = cp.tile([G, BS], F32)
            nc.sync.dma_start(gasT[:], gasT_e[:])
            Msb = cp.tile([NM, W], mybir.dt.float32r)
            nc.gpsimd.dma_start(Msb[:], M_e[:])
            Mbar = cp.tile([NM, 1], F32)
            nc.sync.dma_start(Mbar[:], Mb_e[:])
            nuisT = cp.tile([G, BS], F32)
            nc.sync.dma_start(nuisT[:], nuisT_e[:])
            am8 = cp.tile([G, BS], F32)
            nc.sync.dma_start(am8[:], amr_e[:])
            oh = cp.tile([NI, BS], F32)
            nc.sync.dma_start(oh[:], oh_e[:])
            emb16 = cp.tile([NI, E], F32)
            nc.sync.dma_start(emb16[:], emb_e[:])
            w1a = cp.tile([E, 64], F32)
            nc.sync.dma_start(w1a[:], w1a_e[:])
            w1b = cp.tile([G, 64], F32)
            nc.sync.dma_start(w1b[:], w1b_e[:])
            b1r = cp.tile([1, 64], F32)
            nc.sync.dma_start(b1r[:], b1r_e[:])
            w2s = cp.tile([65, 7], F32)
            nc.sync.dma_start(w2s[:], w2s_e[:])

            ident = cp.tile([TP, TP], F32)
            make_identity(nc, ident[:])
            identB = cp.tile([TP, TP], BF16)
            make_identity(nc, identB[:])

            xmodf = cp.tile([TP, CHW], BF16)  # 0..127 repeated per block (exact in bf16)
            nc.gpsimd.iota(xmodf[:], [[0, CHB], [1, BLK]], channel_multiplier=0,
                           allow_small_or_imprecise_dtypes=True)
            blkio = cp.tile([TP, NBLK], F32)  # 128*r
            nc.gpsimd.iota(blkio[:], [[BLK, NBLK]], channel_multiplier=0,
                           allow_small_or_imprecise_dtypes=True)
            rowoffc = []
            for s in range(NCH):  # p*SEGP + SEGE - SEGW*s
                ro = cp.tile([TP, CHB], I32, tag=f"roff{s}")
                nc.gpsimd.iota(ro[:], [[0, CHB]], base=SEGE - SEGW * s,
                               channel_multiplier=SEGP)
                rowoffc.append(ro)
            kio = cp.tile([TP, K], F32)  # -7..7
            nc.gpsimd.iota(kio[:], [[1, K]], base=-7, channel_multiplier=0,
                           allow_small_or_imprecise_dtypes=True)

            # uT = [gas*am, gas, am, 1]  [18, BS]; uTpre has gas in rows 0:8
            ones = cp.tile([1, BS], F32)
            nc.vector.memset(ones[:], 1.0)
            uT = cp.tile([NM, BS], mybir.dt.float32r)
            nc.gpsimd.dma_start(uT[:], uTp_e[:])
            nc.vector.tensor_tensor(uT[0:G, :], gasT[:], am8[:], OP.mult)
            uTf = cp.tile([NM, BS], F32)
            nc.sync.dma_start(uTf[:], uTp_e[:])
            nc.vector.tensor_tensor(uTf[0:G, :], gasT[:], am8[:], OP.mult)
            negk = cp.tile([TP, 3], F32)
            for k in (1, 2, 3):
                nc.vector.memset(negk[:, k - 1:k], float(-k))

            # embT [128, BS] = emb16.T @ onehotT
            embps = psS.tile([E, BS], F32)
            nc.tensor.matmul(embps[:], emb16[:], oh[:], start=True, stop=True)
            embT = cp.tile([E, BS], F32)
            nc.scalar.activation(embT[:], embps[:], AF.Copy)

            for t in range(NT):
                rs = t * TP  # row offset within the core's 256

                # ---------- row-mean (negated) of differential ----------
                c0ps = psS.tile([TP, 1], F32, tag="mlp")
                nc.tensor.matmul(c0ps[:], uTf[:, rs:rs + TP], Mbar[:],
                                 start=True, stop=True)
                negC0 = pp.tile([TP, 1], F32, tag="negC0")
                nc.scalar.activation(negC0[:], c0ps[:], AF.Copy)

                # ---------- differential -> DRAM scratch (padded) ----------
                for g4 in range(4):
                    d4 = bncp.tile([TP, 2048], F32, tag="d4")
                    for cc in range(4):
                        c = 4 * g4 + cc
                        dps = psA.tile([TP, 512], F32, tag="dps")
                        nc.tensor.matmul(dps[:], uT[:, rs:rs + TP],
                                         Msb[:, 512 * c:512 * (c + 1)],
                                         start=True, stop=True)
                        if (c % 2) == 0:
                            nc.vector.tensor_scalar(
                                d4[:, 512 * cc:512 * (cc + 1)], dps[:],
                                1.0, negC0[:, 0:1], OP.mult, OP.add)
                        else:
                            nc.scalar.activation(d4[:, 512 * cc:512 * (cc + 1)],
                                                 dps[:], AF.Identity,
                                                 bias=negC0[:, 0:1])
                    nc.sync.dma_start(scr[t][g4][:, SEGE:SEGE + SEGW], d4[:])
                    if g4 > 0:
                        nc.scalar.dma_start(scr[t][g4 - 1][:, SEGE + SEGW:SEGP],
                                            d4[:, 0:SEGE])
                    if g4 < NCH - 1:
                        nc.scalar.dma_start(scr[t][g4 + 1][:, 0:SEGE],
                                            d4[:, SEGW - SEGE:SEGW])
                    if g4 == 0:
                        plt = cb.tile([TP, SEGE], F32, tag="pad")
                        nc.vector.tensor_copy(plt[:], _bcast(d4[:, 0:1], SEGE))
                        nc.scalar.dma_start(scr[t][0][:, 0:SEGE], plt[:])
                    if g4 == NCH - 1:
                        prt = cb.tile([TP, SEGE], F32, tag="pad")
                        nc.vector.tensor_copy(prt[:], _bcast(d4[:, SEGW - 1:SEGW],
                                                             SEGE))
                        nc.scalar.dma_start(scr[t][NCH - 1][:, SEGE + SEGW:SEGP],
                                            prt[:])

                # ---------- MLP ----------
                hps = psS.tile([TP, 64], F32, tag="mlp")
                nc.tensor.matmul(hps[:], embT[:, rs:rs + TP], w1a[:],
                                 start=True, stop=False)
                nc.tensor.matmul(hps[:], nuisT[:, rs:rs + TP], w1b[:],
                                 start=False, stop=False)
                nc.tensor.matmul(hps[:], ones[:, rs:rs + TP], b1r[:],
                                 start=False, stop=True)
                h = sp.tile([TP, 64], F32, tag="h")
                nc.scalar.activation(h[:], hps[:], AF.Gelu)
                hTp = psS.tile([64, TP], F32, tag="mlp")
                nc.tensor.transpose(hTp[:], h[:], ident[:])
                hT1 = sp.tile([65, TP], F32, tag="hT1")
                nc.scalar.activation(hT1[0:64, :], hTp[:], AF.Copy)
                nc.vector.memset(hT1[64:65, :], 1.0)
                pps = psS.tile([TP, 7], F32, tag="mlp")
                nc.tensor.matmul(pps[:], hT1[:], w2s[:], start=True, stop=True)
                pv = sp.tile([TP, 7], F32, tag="pv")
                nc.scalar.activation(pv[:], pps[:], AF.Copy)

                # ---------- per-row params ----------
                gain = pp.tile([TP, 1], F32, tag="gain")
                nc.scalar.activation(gain[:], pv[:, 0:1], AF.Exp)
                nc.scalar.activation(gain[:], gain[:], AF.Ln, bias=1.0)
                nc.vector.tensor_scalar(gain[:], gain[:], 0.001, None, OP.add)
                th2 = pp.tile([TP, 1], F32, tag="th2")
                nc.scalar.activation(th2[:], pv[:, 2:3], AF.Tanh)
                th3 = pp.tile([TP, 1], F32, tag="th3")
                nc.scalar.activation(th3[:], pv[:, 3:4], AF.Tanh)
                a_sl = pp.tile([TP, 1], F32, tag="a_sl")  # ws - 1
                nc.vector.tensor_scalar(a_sl[:], th3[:], 0.005, None, OP.mult)
                ws_s = pp.tile([TP, 1], F32, tag="ws_s")  # wscale
                nc.vector.tensor_scalar(ws_s[:], th3[:], 0.005, 1.0, OP.mult, OP.add)
                tsh = pp.tile([TP, 1], F32, tag="tsh")  # index-space shift
                nc.vector.tensor_scalar(tsh[:], th3[:], 0.005 * wl0 / dlam, None,
                                        OP.mult)
                nc.vector.scalar_tensor_tensor(tsh[:], th2[:], 0.05 / dlam, tsh[:],
                                               OP.mult, OP.add)
                lsf = pp.tile([TP, 1], F32, tag="lsf")
                nc.scalar.activation(lsf[:], pv[:, 4:5], AF.Exp)
                nc.scalar.activation(lsf[:], lsf[:], AF.Ln, bias=1.0)
                nc.vector.tensor_scalar(lsf[:], lsf[:], 0.001, 5.0, OP.add, OP.min)
                nc.vector.tensor_scalar(lsf[:], lsf[:], 0.2, 1e-6, OP.max, OP.add)
                linv = pp.tile([TP, 1], F32, tag="linv")
                nc.vector.reciprocal(linv[:], lsf[:])
                st_s = pp.tile([TP, 1], F32, tag="st_s")
                nc.scalar.activation(st_s[:], pv[:, 5:6], AF.Tanh, scale=0.5)
                nc.vector.tensor_scalar(st_s[:], st_s[:], 0.025, 0.025, OP.mult,
                                        OP.add)
                omst = pp.tile([TP, 1], F32, tag="omst")
                nc.vector.tensor_scalar(omst[:], st_s[:], -1.0, 1.0, OP.mult, OP.add)
                nonl = pp.tile([TP, 1], F32, tag="nonl")
                nc.scalar.activation(nonl[:], pv[:, 6:7], AF.Tanh)
                nc.vector.tensor_scalar(nonl[:], nonl[:], 0.02, None, OP.mult)

                # ---------- Gaussian kernel + diagonal weights ----------
                kern = sp.tile([TP, K], F32, tag="kern")
                kernb = sp.tile([TP, K], BF16, tag="kernb")
                nc.vector.tensor_scalar(kern[:], kio[:], linv[:, 0:1], None, OP.mult)
                nc.scalar.activation(kern[:], kern[:], AF.Square)
                nc.scalar.activation(kern[:], kern[:], AF.Exp, scale=-0.5)
                ksum = pp.tile([TP, 1], F32, tag="ksum")
                nc.vector.tensor_reduce(ksum[:], kern[:], mybir.AxisListType.X,
                                        OP.add)
                krec = pp.tile([TP, 1], F32, tag="krec")
                nc.vector.reciprocal(krec[:], ksum[:])
                nc.vector.tensor_scalar(kernb[:], kern[:], krec[:, 0:1], None,
                                        OP.mult)
                diag = mp1.tile([TP, K * TP], BF16, tag="diag")
                for k in range(K):
                    nc.vector.tensor_tensor(diag[:, TP * k:TP * (k + 1)], identB[:],
                                            _bcast(kernb[:, k:k + 1], TP), OP.mult)

                # ---------- gather indices ----------
                pst = sp.tile([TP, NBLK], F32, tag="pst")
                nc.vector.scalar_tensor_tensor(
                    pst[:], blkio[:], ws_s[:, 0:1],
                    _bcast(tsh[:, 0:1], NBLK),
                    OP.mult, OP.add)
                bfl = sp.tile([TP, NBLK], F32, tag="bfl")
                nc.vector.tensor_scalar(bfl[:], pst[:], 170.5, None, OP.add)
                bi = pp.tile([TP, NBLK], I32, tag="bi")
                nc.vector.tensor_copy(bi[:], bfl[:])
                nc.vector.tensor_scalar(bi[:], bi[:], 172, None, OP.subtract)
                bf2 = sp.tile([TP, NBLK], F32, tag="bf2")
                nc.vector.tensor_copy(bf2[:], bi[:])
                phi = sp.tile([TP, NBLK], F32, tag="phi")
                nc.vector.tensor_tensor(phi[:], pst[:], bf2[:], OP.subtract)
                phib = sp.tile([TP, NBLK], BF16, tag="phib")
                nc.vector.tensor_copy(phib[:], phi[:])
                idx = sp.tile([TP, NBLK], I32, tag="idx")
                nc.vector.tensor_tensor(idx[:], rowoff[:], bi[:], OP.add)

                Rpad = bigp.tile([TP, 2 * 8 + W], BF16, tag="Rpad")

                # ---------- per-chunk gather + resample ----------
                for c in range(NCH):
                    idxc = sp.tile([TP, CHB], I32, tag="idxc")
                    nc.vector.tensor_tensor(idxc[:], rowoffc[c][:],
                                            bi[:, CHB * c:CHB * (c + 1)], OP.add)
                    win = mp.tile([TP, CHB * LWIN], BF16, tag="win")
                    for rr in range(CHB):
                        nc.gpsimd.indirect_dma_start(
                            out=win[:, LWIN * rr:LWIN * (rr + 1)],
                            out_offset=None,
                            in_=scr[t][c][:],
                            in_offset=bass.IndirectOffsetOnAxis(
                                ap=idxc[:, rr:rr + 1], axis=1),
                        )
                    t1 = cb.tile([TP, CHB * LWIN], BF16, tag="t1")
                    nc.vector.tensor_tensor(t1[:, 0:CHB * LWIN - 1],
                                            win[:, 1:], win[:, :-1], OP.subtract)
                    d2 = cb.tile([TP, CHB * LWIN], BF16, tag="d2")
                    nc.vector.tensor_tensor(d2[:, 1:CHB * LWIN - 1],
                                            t1[:, 1:CHB * LWIN - 1],
                                            t1[:, 0:CHB * LWIN - 2], OP.subtract)

                    rho = cb.tile([TP, CHW], BF16, tag="rho")
                    nc.vector.scalar_tensor_tensor(
                        rho[:].rearrange("p (b x) -> p b x", x=BLK),
                        xmodf[:].rearrange("p (b x) -> p b x", x=BLK),
                        a_sl[:, 0:1],
                        phib[:, CHB * c:CHB * (c + 1)].unsqueeze(-1)
                        .to_broadcast([TP, CHB, BLK]),
                        OP.mult, OP.add)

                    def wap(tl, off):
                        return tl[:].rearrange("p (b l) -> p b l", l=LWIN)[
                            :, :, off:off + BLK]

                    # acc = win[x] + t1[x]*rho + sum_k relu(rho-k)*d2[x+k]
                    def c3(ap):
                        return ap.rearrange("p (b x) -> p b x", x=BLK)

                    t0 = cb.tile([TP, CHW], BF16, tag="pk")
                    nc.vector.scalar_tensor_tensor(c3(t0[:]), c3(rho[:]), 1.0,
                                                   wap(t1, 0), OP.mult, OP.mult)
                    acc = mp.tile([TP, CHW], BF16, tag="acc")
                    nc.vector.tensor_tensor(c3(acc[:]), wap(win, 0), c3(t0[:]),
                                            OP.add)
                    for k in (1, 2):
                        rk = cb.tile([TP, CHW], BF16, tag="rk")
                        nc.scalar.activation(rk[:], rho[:], AF.Relu,
                                             bias=negk[:, k - 1:k])
                        pk = cb.tile([TP, CHW], BF16, tag="pk")
                        nc.vector.scalar_tensor_tensor(c3(pk[:]), c3(rk[:]), 1.0,
                                                       wap(d2, k), OP.mult, OP.mult)
                        if k == 1:
                            acc2 = mp.tile([TP, CHW], BF16, tag="acc")
                            nc.vector.tensor_tensor(acc2[:], acc[:], pk[:], OP.add)
                            acc = acc2
                        else:
                            nc.vector.tensor_tensor(
                                Rpad[:, 8 + CHW * c:8 + CHW * (c + 1)],
                                acc[:], pk[:], OP.add)
                    if c == 0:
                        nc.scalar.activation(Rpad[:, 1:8],
                                             _bcast(Rpad[:, 8:9], 7), AF.Copy)
                    if c == NCH - 1:
                        nc.scalar.activation(Rpad[:, 8 + W:15 + W],
                                             _bcast(Rpad[:, 7 + W:8 + W], 7),
                                             AF.Copy)


                # ---------- conv (PE) + chunked epilogue ----------
                nl = bigp2.tile([TP, W], BF16, tag="nl")
                msum = sp.tile([TP, 16], F32, tag="msum")
                for c in reversed(range(16)):
                    cps = psC.tile([TP, 512], F32, tag="cps")
                    for k in range(K):
                        nc.tensor.matmul(cps[:], diag[:, TP * k:TP * (k + 1)],
                                         Rpad[:, 1 + 512 * c + k:1 + 512 * c + k + 512],
                                         start=(k == 0), stop=(k == K - 1))
                    cnt = cb.tile([TP, 512], F32, tag="cnt")
                    nc.scalar.activation(cnt[:], cps[:], AF.Exp, scale=-1.0,
                                         bias=negC0[:, 0:1])
                    pst_c = bncp.tile([TP, 512], BF16, tag="pst_c")
                    nc.scalar.activation(pst_c[:], cnt[:], AF.Identity,
                                         bias=pv[:, 1:2], scale=gain[:, 0:1])
                    sq_c = bncp.tile([TP, 512], BF16, tag="sq_c")
                    nc.scalar.activation(sq_c[:], pst_c[:], AF.Square)
                    nc.vector.scalar_tensor_tensor(
                        nl[:, 512 * c:512 * (c + 1)], sq_c[:], nonl[:, 0:1],
                        pst_c[:], OP.mult, OP.add, accum_out=msum[:, c:c + 1])
                mtot = pp.tile([TP, 1], F32, tag="mtot")
                nc.vector.tensor_reduce(mtot[:], msum[:], mybir.AxisListType.X,
                                        OP.add)
                stm = pp.tile([TP, 1], F32, tag="stm")
                nc.vector.tensor_scalar(stm[:], mtot[:], st_s[:, 0:1], 1.0 / W,
                                        OP.mult, OP.mult)
                for g4 in range(4):
                    fin = bncp.tile([TP, 2048], F32, tag="fin")
                    nc.scalar.activation(fin[:], nl[:, 2048 * g4:2048 * (g4 + 1)],
                                         AF.Identity, bias=stm[:, 0:1],
                                         scale=omst[:, 0:1])
                    nc.sync.dma_start(out_e[rs:rs + TP, 2048 * g4:2048 * (g4 + 1)],
                                      fin[:])

    nc.compile()
    return nc


def _prep(inputs):
    gas = np.asarray(inputs["gas_columns"], np.float32)
    ids = np.asarray(inputs["instrument_ids"]).astype(np.int64)
    nuis = np.asarray(inputs["nuisance_latent"], np.float32)
    am = np.asarray(inputs["air_mass"], np.float32)
    wl = np.asarray(inputs["wavelengths_nm"], np.float32)
    ab = np.asarray(inputs["absorption"], np.float32)
    cb = np.asarray(inputs["continuum_basis"], np.float32)
    ray = np.asarray(inputs["rayleigh_od"], np.float32)
    emb = np.asarray(inputs["embed_table"], np.float32)
    w1 = np.asarray(inputs["w1"], np.float32)
    b1 = np.asarray(inputs["b1"], np.float32)
    w2 = np.asarray(inputs["w2"], np.float32)
    b2 = np.asarray(inputs["b2"], np.float32)

    wl0 = float(wl[0])
    dlam = float((wl[-1] - wl[0]) / (W - 1))

    Mmat = np.concatenate(
        [ab, cb[0:G], ray[None, :], cb[G:G + 1]], axis=0).astype(np.float32)
    Mbar = (-Mmat.mean(axis=1, keepdims=True)).astype(np.float32)
    w1a = np.ascontiguousarray(w1[G:G + E])            # embed rows
    w1b = np.ascontiguousarray(w1[0:G])                # nuisance rows
    b1r = b1[None, :].astype(np.float32)
    w2s = np.concatenate([w2, b2[None, :]], 0)

    in_maps = []
    for c in range(NCORES):
        r = slice(c * BS, (c + 1) * BS)
        oh = (ids[r][None, :] == np.arange(NI)[:, None]).astype(np.float32)
        gT = np.ascontiguousarray(gas[r].T)
        uTpre = np.concatenate([gT, gT, am[r][None, :],
                                np.ones((1, BS), np.float32)], 0)
        in_maps.append({
            "gasT": gT,
            "uTpre": uTpre,
            "amr8": np.repeat(am[r][None, :], G, axis=0),
            "nuisT": np.ascontiguousarray(nuis[r].T),
            "am": np.ascontiguousarray(am[r][None, :]),
            "onehotT": oh,
            "embT16": emb,
            "Mmat": Mmat,
            "Mbar": Mbar,
            "w1a": w1a,
            "w1b": w1b,
            "b1r": b1r,
            "w2s": w2s,
        })
    return in_maps, wl0, dlam


def kernel(**inputs):
    in_maps, wl0, dlam = _prep(inputs)
    key = (round(wl0, 6), round(dlam, 9))
    if key not in _CACHE:
        _CACHE[key] = _build(wl0, dlam)
    nc = _CACHE[key]
    res = run_bass_kernel_spmd(nc, in_maps, core_ids=list(range(NCORES)))
    outs = [res.results[i]["out"] for i in range(NCORES)]
    return np.concatenate(outs, axis=0).astype(np.float32)


if __name__ == "__main__":
    rng = np.random.default_rng(0)
    ins = {
        "gas_columns": rng.random((B, G), dtype=np.float32),
        "instrument_ids": rng.integers(0, NI, B),
        "nuisance_latent": rng.standard_normal((B, G)).astype(np.float32),
        "air_mass": 1 + 2 * rng.random(B, dtype=np.float32),
        "wavelengths_nm": np.linspace(300, 400, W, dtype=np.float32),
        "absorption": 0.1 * rng.random((G, W), dtype=np.float32),
        "continuum_basis": 0.05 * rng.standard_normal((G + 1, W)).astype(np.float32),
        "rayleigh_od": rng.random(W, dtype=np.float32),
        "embed_table": rng.standard_normal((NI, E)).astype(np.float32),
        "w1": rng.standard_normal((G + E, 64)).astype(np.float32) / 12,
        "b1": np.zeros(64, np.float32),
        "w2": rng.standard_normal((64, 7)).astype(np.float32) / 8,
        "b2": np.zeros(7, np.float32),
    }
    out = kernel(**ins)
    print("out", out.shape, out.dtype, np.abs(out).mean())
